# revision 17
# baseline (speedup 1.0000x reference)
"""Trainium2 Bass kernel for nn_MultiHeadAttention_53309134078537.

Reference computation (B=4, S=2048, D=512, H=8, HD=64):
    q = split_heads(Q @ wq + b); k = split_heads(K @ wq + b); v = split_heads(V @ wq + b)
    logits = (q @ k^T) / 8 + pad_mask * (-1e9)
    attn = softmax(logits)          # (B, H, S, S) -- 512 MB fp32, dominates memory traffic
    z = attn @ v; out = merge(z) @ out_kernel + out_bias
    returns (out, attn)

Sharding: 8 cores = (batch b = c//2) x (head-group hg = c%2, 4 heads each).

Per core the attention is computed transposed (logitsT[k, q]): the padding mask becomes a
per-partition ACT bias and the z matmul consumes attn^T directly (no on-chip transpose of
the 16.8M-element attention matrix). The K=64 logits matmuls of an even/odd head pair are
row-packed into the two halves of the PE array (tile_position (0,0)/(64,0)) so both heads'
logits stream concurrently. Softmax sums ride along as a ones-column in the v operand of
the z matmul; 1/sum is computed by DVE reciprocal in partition layout (no ACT table
switches), broadcast across partitions via a small DRAM bounce, and applied in-place by
DVE in 16-bit 2x mode; z is renormalized by the same broadcast row. Work is pipelined in
superunits (head-pair x q-half, k=2048, q=1024) so the z -> 1/sum -> normalize tail hides
behind the next superunit's logits/exp.

attn is written to HBM as fp16 [k, q]; the host transposes to [q, k] and casts to fp32
(pure data movement). The out-projection is emitted per head-pair as separate partials;
the host sums the four partials per batch (out_bias rides on one of them, zeros on the
hg=1 cores).
"""

import numpy as np

B, S, D, H, HD = 4, 2048, 512, 8, 64
HPC = 4            # heads per core
DHG = HPC * HD     # 256: d_out slice per core
NCORES = 8
SCALE = 1.0 / 8.0
NEG = -1e9 * SCALE  # mask bias applied after the activation scale

P = 128
ST = S // P        # 16 tiles of 128 along sequence (k)
QH = S // 2        # 1024: q extent of one superunit
DI = D // P        # 4 tiles of 128 along d_in
DO2 = DHG // P     # 2 tiles of 128 along the core's d_out slice

_CACHE = {}


def _build():
    import concourse.bass as bass
    import concourse.tile as tile
    from concourse import bacc, mybir

    f32, f16 = mybir.dt.float32, mybir.dt.float16
    AF = mybir.ActivationFunctionType
    ALU = mybir.AluOpType

    nc = bacc.Bacc("TRN2", target_bir_lowering=False)

    Q = nc.dram_tensor("q_in", [S, D], f32, kind="ExternalInput")
    K = nc.dram_tensor("k_in", [S, D], f32, kind="ExternalInput")
    V = nc.dram_tensor("v_in", [S, D], f32, kind="ExternalInput")
    MASK = nc.dram_tensor("mask", [1, S], f32, kind="ExternalInput")
    WQ = nc.dram_tensor("wq", [D, DHG], f32, kind="ExternalInput")
    WQB = nc.dram_tensor("wqb", [1, DHG], f32, kind="ExternalInput")
    WO = nc.dram_tensor("wo", [DHG, D], f32, kind="ExternalInput")
    WOB = nc.dram_tensor("wob", [1, D], f32, kind="ExternalInput")
    ATTN = nc.dram_tensor("attn_t", [HPC, S, S], f16, kind="ExternalOutput")
    OUT = nc.dram_tensor("out_p", [DO2, S, D], f32, kind="ExternalOutput")

    with tile.TileContext(nc) as tc:
        with (
            tc.tile_pool(name="persist", bufs=1) as persist,
            tc.tile_pool(name="dram", bufs=1, space="DRAM") as dram,
            tc.tile_pool(name="psL", bufs=3, space="PSUM") as psL,
            tc.tile_pool(name="psZ", bufs=2, space="PSUM") as psZ,
            tc.tile_pool(name="work", bufs=2) as work,
        ):
            # ---- persistent SBUF state ----
            qT = persist.tile([P, DO2, S], f16)      # q_projT: [dout, s]
            kT = persist.tile([P, DO2, S], f16)      # k_projT
            zT = persist.tile([P, DO2, S], f16)      # zT: rows h*64..h*64+64 per head
            vext = persist.tile([P, ST, HPC, HD + 1], f16)  # v_proj + ones column
            wo_sb = persist.tile([P, DO2, D], f16)
            mask_bias = persist.tile([P, ST], f32)   # NEG * mask, partition layout
            wqb_part = persist.tile([P, DO2], f32)   # wq bias, partition layout
            wqb_bc = persist.tile([P, DHG], f16)     # wq bias broadcast along partitions
            wob_bc = persist.tile([P, D], f16)       # out bias broadcast along partitions

            with tc.tile_pool(name="load", bufs=1) as load:
                # fp16 copies of Q/K/V in DRAM (xbar transpose is 16-bit only),
                # cast per 128-column block so each transpose can start as soon
                # as its block lands.
                x16s = [
                    dram.tile([S, D], f16, tag="x16", bufs=3, name=f"x16_{i}")
                    for i in range(3)
                ]
                for i, src in enumerate((Q, K, V)):
                    for t in range(DI):
                        nc.gpsimd.dma_start(
                            out=x16s[i][:, t * P : (t + 1) * P],
                            in_=src.ap()[:, t * P : (t + 1) * P],
                        )

                # weights via the HWDGE queues + DVE cast (keeps SWDGE free)
                wq_f32 = load.tile([P, DI, DHG], f32)
                nc.sync.dma_start(
                    out=wq_f32, in_=WQ.ap().rearrange("(t p) n -> p t n", p=P)
                )
                wq_sb = load.tile([P, DI, DHG], f16)
                nc.vector.tensor_copy(out=wq_sb, in_=wq_f32)
                wo_f32 = load.tile([P, DO2, D], f32)
                nc.scalar.dma_start(
                    out=wo_f32, in_=WO.ap().rearrange("(t p) n -> p t n", p=P)
                )
                nc.vector.tensor_copy(out=wo_sb, in_=wo_f32)
                nc.sync.dma_start(
                    out=wqb_part, in_=WQB.ap().rearrange("1 (t p) -> p t", p=P)
                )
                nc.gpsimd.dma_start(out=wqb_bc, in_=WQB.ap().to_broadcast((P, DHG)))
                nc.gpsimd.dma_start(out=wob_bc, in_=WOB.ap().to_broadcast((P, D)))

                mask_part = load.tile([P, ST], f32)
                nc.sync.dma_start(
                    out=mask_part, in_=MASK.ap().rearrange("1 (t p) -> p t", p=P)
                )
                nc.vector.tensor_scalar_mul(out=mask_bias, in0=mask_part, scalar1=NEG)

                nc.vector.memset(vext, 0.0)

                # Per tensor: xbar-transpose the fp16 copy to X^T in SBUF, then
                # project. q/k produce [dout, s]; v lands in per-head v_ext tiles.
                for i, dst in ((0, qT), (1, kT), (2, None)):
                    xTsb = load.tile([P, DI, S], f16, tag="xT", bufs=2, name=f"xT{i}")
                    for t in range(DI):
                        nc.sync.dma_start_transpose(
                            out=xTsb[:, t, :], in_=x16s[i][:, t * P : (t + 1) * P]
                        )
                    if dst is not None:
                        for dot in range(DO2):
                            for qc in range(S // 512):
                                ps = psL.tile([P, 1024], f32, tag="l")
                                for di in range(DI):
                                    nc.tensor.matmul(
                                        ps[:, :512],
                                        lhsT=wq_sb[:, di, dot * P : (dot + 1) * P],
                                        rhs=xTsb[:, di, qc * 512 : (qc + 1) * 512],
                                        start=(di == 0),
                                        stop=(di == DI - 1),
                                    )
                                nc.vector.tensor_scalar(
                                    out=dst[:, dot, qc * 512 : (qc + 1) * 512],
                                    in0=ps[:, :512],
                                    scalar1=wqb_part[:, dot : dot + 1],
                                    scalar2=None,
                                    op0=ALU.add,
                                )
                    else:
                        for st in range(ST):
                            ps = psL.tile([P, 1024], f32, tag="l")
                            for di in range(DI):
                                nc.tensor.matmul(
                                    ps[:, :DHG],
                                    lhsT=xTsb[:, di, st * P : (st + 1) * P],
                                    rhs=wq_sb[:, di, :],
                                    start=(di == 0),
                                    stop=(di == DI - 1),
                                )
                            for h in range(HPC):
                                nc.vector.tensor_add(
                                    out=vext[:, st, h, :HD],
                                    in0=ps[:, h * HD : (h + 1) * HD],
                                    in1=wqb_bc[:, h * HD : (h + 1) * HD],
                                )
                        nc.vector.memset(vext[:, :, :, HD : HD + 1], 1.0)

            # ---- attention: 4 superunits (head-pair x q-half), pipelined ----
            with tc.tile_pool(name="exp", bufs=52) as exp_pool:
                su_state = {}

                def emit_logits_exp(su):
                    """Row-packed logits + exp for both heads of the pair."""
                    dot, qh = su // 2, su % 2
                    q0 = qh * QH
                    tA, tB = [], []
                    for kt in range(ST):
                        psA = psL.tile([P, 1024], f32, tag="l", name=f"psA_{su}_{kt}")
                        psB = psL.tile([P, 1024], f32, tag="l", name=f"psB_{su}_{kt}")
                        for j in range(2):
                            qs = slice(q0 + j * 512, q0 + (j + 1) * 512)
                            os_ = slice(j * 512, (j + 1) * 512)
                            nc.tensor.matmul(
                                psA[:, os_],
                                lhsT=kT[0:HD, dot, kt * P : (kt + 1) * P],
                                rhs=qT[0:HD, dot, qs],
                                start=True,
                                stop=True,
                                tile_position=(0, 0),
                            )
                            nc.tensor.matmul(
                                psB[:, os_],
                                lhsT=kT[HD : 2 * HD, dot, kt * P : (kt + 1) * P],
                                rhs=qT[HD : 2 * HD, dot, qs],
                                start=True,
                                stop=True,
                                tile_position=(HD, 0),
                            )
                        for tl, ps, name in ((tA, psA, "A"), (tB, psB, "B")):
                            et = exp_pool.tile(
                                [P, QH], f16, tag="expT", name=f"e{name}_{su}_{kt}"
                            )
                            tl.append(et)
                            nc.scalar.activation(
                                out=et,
                                in_=ps,
                                func=AF.Exp,
                                bias=mask_bias[:, kt : kt + 1],
                                scale=SCALE,
                            )
                    su_state[su] = (tA, tB)

                def emit_tail(su, side):
                    """z + 1/sum + normalize + writeback for one head of su."""
                    dot, qh = su // 2, su % 2
                    h = 2 * dot + side
                    r0 = side * HD
                    q0 = qh * QH
                    tiles = su_state[su][side]

                    zext = work.tile([P, QH], f32, tag="zext")
                    for qc in range(2):
                        ps = psZ.tile([P, 512], f32, tag="z")
                        for kt in range(ST):
                            nc.tensor.matmul(
                                ps[: HD + 1, :],
                                lhsT=vext[:, kt, h, :],
                                rhs=tiles[kt][:, qc * 512 : (qc + 1) * 512],
                                start=(kt == 0),
                                stop=(kt == ST - 1),
                            )
                        nc.vector.tensor_copy(
                            out=zext[: HD + 1, qc * 512 : (qc + 1) * 512],
                            in_=ps[: HD + 1, :],
                        )
                    # 1/sum on DVE in partition layout (no ACT tables):
                    # [1,1024] -> [128,8] -> reciprocal -> DRAM row -> broadcast
                    sum_dram = dram.tile([1, QH], f32, tag="sum_d", bufs=3)
                    nc.sync.dma_start(out=sum_dram, in_=zext[HD : HD + 1, :])
                    rp = work.tile([P, 8], f32, tag="rp")
                    nc.gpsimd.dma_start(
                        out=rp, in_=sum_dram.rearrange("1 (c p) -> p c", p=P)
                    )
                    nc.vector.reciprocal(out=rp, in_=rp)
                    rsum_dram = dram.tile([1, QH], f32, tag="rsum_d", bufs=3)
                    nc.sync.dma_start(
                        out=rsum_dram.rearrange("1 (c p) -> p c", p=P), in_=rp
                    )
                    rbc = work.tile([P, QH], f16, tag="rbc", bufs=3)
                    nc.gpsimd.dma_start(out=rbc, in_=rsum_dram.to_broadcast((P, QH)))

                    for kt in range(ST):
                        nc.vector.tensor_mul(out=tiles[kt], in0=tiles[kt], in1=rbc)
                        nc.sync.dma_start(
                            out=ATTN.ap()[h, kt * P : (kt + 1) * P, q0 : q0 + QH],
                            in_=tiles[kt],
                        )
                    nc.vector.tensor_mul(
                        out=zT[r0 : r0 + HD, dot, q0 : q0 + QH],
                        in0=zext[:HD, :],
                        in1=rbc[:HD, :],
                    )
                    if side == 1:
                        su_state.pop(su)

                def emit_out(dt):
                    """Partial out-projection for head pair dt (zT tile dt)."""
                    for qt in range(ST):
                        ps = psL.tile([P, 1024], f32, tag="l")
                        nc.tensor.matmul(
                            ps[:, :512],
                            lhsT=zT[:, dt, qt * P : (qt + 1) * P],
                            rhs=wo_sb[:, dt, :],
                            start=True,
                            stop=True,
                        )
                        osb = work.tile([P, D], f32, tag="osb")
                        if dt == 0:
                            nc.vector.tensor_add(out=osb, in0=ps[:, :512], in1=wob_bc)
                        else:
                            nc.vector.tensor_copy(out=osb, in_=ps[:, :512])
                        nc.sync.dma_start(
                            out=OUT.ap()[dt, qt * P : (qt + 1) * P, :], in_=osb
                        )

                for su in range(4):
                    emit_logits_exp(su)
                    if su > 0:
                        emit_tail(su - 1, 0)
                        emit_tail(su - 1, 1)
                    if su == 2:
                        emit_out(0)  # heads 0/1 fully done after su=1 tails
                emit_tail(3, 0)
                emit_tail(3, 1)
                emit_out(1)

    nc.finalize()
    return nc


def kernel(Q, K, V, pad_mask, wq_kernel, wq_bias, out_kernel, out_bias, **run_kwargs):
    from concourse.bass_utils import run_bass_kernel_spmd

    if "nc" not in _CACHE:
        _CACHE["nc"] = _build()
    nc = _CACHE["nc"]

    in_maps = []
    for c in range(NCORES):
        b, hg = c // 2, c % 2
        hs = slice(hg * DHG, (hg + 1) * DHG)
        in_maps.append(
            {
                "q_in": np.ascontiguousarray(Q[b], dtype=np.float32),
                "k_in": np.ascontiguousarray(K[b], dtype=np.float32),
                "v_in": np.ascontiguousarray(V[b], dtype=np.float32),
                "mask": np.ascontiguousarray(
                    pad_mask[b, 0, 0, :][None, :], dtype=np.float32
                ),
                "wq": np.ascontiguousarray(wq_kernel[:, hs], dtype=np.float32),
                "wqb": np.ascontiguousarray(wq_bias[hs][None, :], dtype=np.float32),
                "wo": np.ascontiguousarray(out_kernel[hs, :], dtype=np.float32),
                "wob": np.ascontiguousarray(
                    (out_bias if hg == 0 else np.zeros_like(out_bias))[None, :],
                    dtype=np.float32,
                ),
            }
        )

    res = run_bass_kernel_spmd(nc, in_maps, core_ids=list(range(NCORES)), **run_kwargs)
    results = res.results if hasattr(res, "results") else res

    out = np.empty((B, S, D), dtype=np.float32)
    attn = np.empty((B, H, S, S), dtype=np.float32)
    for c in range(NCORES):
        b, hg = c // 2, c % 2
        at = results[c]["attn_t"]  # fp16 [HPC, S(k), S(q)]
        for i in range(HPC):
            attn[b, hg * HPC + i] = at[i].T
    for b in range(B):
        out[b] = (
            results[2 * b]["out_p"].sum(axis=0) + results[2 * b + 1]["out_p"].sum(axis=0)
        )
    if "trace" in run_kwargs:
        _CACHE["last_run"] = res
    return out, attn


# revision 21
# speedup vs baseline: 1.1343x; 1.1343x over previous
"""Trainium2 Bass kernel for nn_MultiHeadAttention_53309134078537.

Reference computation (B=4, S=2048, D=512, H=8, HD=64):
    q = split_heads(Q @ wq + b); k = split_heads(K @ wq + b); v = split_heads(V @ wq + b)
    logits = (q @ k^T) / 8 + pad_mask * (-1e9)
    attn = softmax(logits)          # (B, H, S, S) -- 512 MB fp32, dominates memory traffic
    z = attn @ v; out = merge(z) @ out_kernel + out_bias
    returns (out, attn)

Sharding: 8 cores = (batch b = c//2) x (head-group hg = c%2, 4 heads each).

Per core the attention is computed transposed (logitsT[k, q]): the padding mask becomes a
per-partition ACT bias and the z matmul consumes attn^T directly (no on-chip transpose of
the 16.8M-element attention matrix). The K=64 logits matmuls of an even/odd head pair are
row-packed into the two halves of the PE array (tile_position (0,0)/(64,0)) so both heads'
logits stream concurrently. Softmax sums ride along as a ones-column in the v operand of
the z matmul; 1/sum is computed by DVE reciprocal in partition layout (no ACT table
switches), broadcast across partitions via a small DRAM bounce, and applied in-place by
DVE in 16-bit 2x mode; z is renormalized by the same broadcast row. Work is pipelined in
superunits (head-pair x q-half, k=2048, q=1024) so the z -> 1/sum -> normalize tail hides
behind the next superunit's logits/exp.

attn is written to HBM as fp16 [k, q]; the host transposes to [q, k] and casts to fp32
(pure data movement). The out-projection is emitted per head-pair as separate partials;
the host sums the four partials per batch (out_bias rides on one of them, zeros on the
hg=1 cores).
"""

import numpy as np

B, S, D, H, HD = 4, 2048, 512, 8, 64
HPC = 4            # heads per core
DHG = HPC * HD     # 256: d_out slice per core
NCORES = 8
SCALE = 1.0 / 8.0
NEG = -1e9 * SCALE  # mask bias applied after the activation scale

P = 128
ST = S // P        # 16 tiles of 128 along sequence (k)
QH = S // 2        # 1024: q extent of one superunit
DI = D // P        # 4 tiles of 128 along d_in
DO2 = DHG // P     # 2 tiles of 128 along the core's d_out slice

_CACHE = {}


def _build():
    import concourse.bass as bass
    import concourse.tile as tile
    from concourse import bacc, mybir

    f32, f16 = mybir.dt.float32, mybir.dt.float16
    AF = mybir.ActivationFunctionType
    ALU = mybir.AluOpType

    nc = bacc.Bacc("TRN2", target_bir_lowering=False)

    Q = nc.dram_tensor("q_in", [S, D], f32, kind="ExternalInput")
    K = nc.dram_tensor("k_in", [S, D], f32, kind="ExternalInput")
    V = nc.dram_tensor("v_in", [S, D], f32, kind="ExternalInput")
    MASK = nc.dram_tensor("mask", [1, S], f32, kind="ExternalInput")
    WQ = nc.dram_tensor("wq", [D, DHG], f32, kind="ExternalInput")
    WQB = nc.dram_tensor("wqb", [1, DHG], f32, kind="ExternalInput")
    WO = nc.dram_tensor("wo", [DHG, D], f32, kind="ExternalInput")
    WOB = nc.dram_tensor("wob", [1, D], f32, kind="ExternalInput")
    ATTN = nc.dram_tensor("attn_t", [HPC, S, S], f16, kind="ExternalOutput")
    OUT = nc.dram_tensor("out_p", [DO2, S, D], f32, kind="ExternalOutput")

    with tile.TileContext(nc) as tc:
        with (
            tc.tile_pool(name="persist", bufs=1) as persist,
            tc.tile_pool(name="dram", bufs=1, space="DRAM") as dram,
            tc.tile_pool(name="psL", bufs=3, space="PSUM") as psL,
            tc.tile_pool(name="psZ", bufs=2, space="PSUM") as psZ,
            tc.tile_pool(name="work", bufs=2) as work,
        ):
            # ---- persistent SBUF state ----
            qT = persist.tile([P, DO2, S], f16)      # q_projT: [dout, s]
            kT = persist.tile([P, DO2, S], f16)      # k_projT
            zT = persist.tile([P, DO2, S], f16)      # zT: rows h*64..h*64+64 per head
            vext = persist.tile([P, ST, HPC, HD + 1], f16)  # v_proj + ones column
            wo_sb = persist.tile([P, DO2, D], f16)
            mask_bias = persist.tile([P, ST], f32)   # NEG * mask, partition layout
            wqb_part = persist.tile([P, DO2], f32)   # wq bias, partition layout
            wqb_bc = persist.tile([P, DHG], f16)     # wq bias broadcast along partitions
            wob_bc = persist.tile([P, D], f16)       # out bias broadcast along partitions

            with tc.tile_pool(name="load", bufs=1) as load:
                # fp16 copies of Q/K/V in DRAM (xbar transpose is 16-bit only),
                # cast per 128-column block so each transpose can start as soon
                # as its block lands.
                x16s = [
                    dram.tile([S, D], f16, tag="x16", bufs=3, name=f"x16_{i}")
                    for i in range(3)
                ]
                # K first: the first logits matmul needs kT+qT; V is consumed
                # latest (first z).
                for i, src in ((1, K), (0, Q), (2, V)):
                    nc.gpsimd.dma_start(out=x16s[i], in_=src.ap())

                # weights via the HWDGE queues + DVE cast (keeps SWDGE free)
                wq_f32 = load.tile([P, DI, DHG], f32)
                nc.sync.dma_start(
                    out=wq_f32, in_=WQ.ap().rearrange("(t p) n -> p t n", p=P)
                )
                wq_sb = load.tile([P, DI, DHG], f16)
                nc.vector.tensor_copy(out=wq_sb, in_=wq_f32)
                wo_f32 = load.tile([P, DO2, D], f32)
                nc.scalar.dma_start(
                    out=wo_f32, in_=WO.ap().rearrange("(t p) n -> p t n", p=P)
                )
                nc.vector.tensor_copy(out=wo_sb, in_=wo_f32)
                nc.sync.dma_start(
                    out=wqb_part, in_=WQB.ap().rearrange("1 (t p) -> p t", p=P)
                )
                nc.gpsimd.dma_start(out=wqb_bc, in_=WQB.ap().to_broadcast((P, DHG)))
                nc.gpsimd.dma_start(out=wob_bc, in_=WOB.ap().to_broadcast((P, D)))

                mask_part = load.tile([P, ST], f32)
                nc.sync.dma_start(
                    out=mask_part, in_=MASK.ap().rearrange("1 (t p) -> p t", p=P)
                )
                nc.vector.tensor_scalar_mul(out=mask_bias, in0=mask_part, scalar1=NEG)

                nc.vector.memset(vext, 0.0)

                # Per tensor: xbar-transpose the fp16 copy to X^T in SBUF, then
                # project. q/k produce [dout, s]; v lands in per-head v_ext tiles.
                for i, dst in ((1, kT), (0, qT), (2, None)):
                    xTsb = load.tile([P, DI, S], f16, tag="xT", bufs=2, name=f"xT{i}")
                    for t in range(DI):
                        nc.sync.dma_start_transpose(
                            out=xTsb[:, t, :], in_=x16s[i][:, t * P : (t + 1) * P]
                        )
                    if dst is not None:
                        for dot in range(DO2):
                            for qc in range(S // 512):
                                ps = psL.tile([P, 1024], f32, tag="l")
                                for di in range(DI):
                                    nc.tensor.matmul(
                                        ps[:, :512],
                                        lhsT=wq_sb[:, di, dot * P : (dot + 1) * P],
                                        rhs=xTsb[:, di, qc * 512 : (qc + 1) * 512],
                                        start=(di == 0),
                                        stop=(di == DI - 1),
                                    )
                                nc.scalar.activation(
                                    out=dst[:, dot, qc * 512 : (qc + 1) * 512],
                                    in_=ps[:, :512],
                                    func=AF.Identity,
                                    bias=wqb_part[:, dot : dot + 1],
                                    scale=1.0,
                                )
                    else:
                        for st in range(ST):
                            ps = psL.tile([P, 1024], f32, tag="l")
                            for di in range(DI):
                                nc.tensor.matmul(
                                    ps[:, :DHG],
                                    lhsT=xTsb[:, di, st * P : (st + 1) * P],
                                    rhs=wq_sb[:, di, :],
                                    start=(di == 0),
                                    stop=(di == DI - 1),
                                )
                            for h in range(HPC):
                                nc.vector.tensor_add(
                                    out=vext[:, st, h, :HD],
                                    in0=ps[:, h * HD : (h + 1) * HD],
                                    in1=wqb_bc[:, h * HD : (h + 1) * HD],
                                )
                        nc.vector.memset(vext[:, :, :, HD : HD + 1], 1.0)

            # ---- attention: 4 superunits (head-pair x q-half), pipelined ----
            with tc.tile_pool(name="exp", bufs=70) as exp_pool:
                su_state = {}

                def emit_logits_exp(su):
                    """Row-packed logits + exp for both heads of the pair."""
                    dot, qh = su // 2, su % 2
                    q0 = qh * QH
                    tA, tB = [], []
                    for kt in range(ST):
                        psA = psL.tile([P, 1024], f32, tag="l", name=f"psA_{su}_{kt}")
                        psB = psL.tile([P, 1024], f32, tag="l", name=f"psB_{su}_{kt}")
                        for j in range(2):
                            qs = slice(q0 + j * 512, q0 + (j + 1) * 512)
                            os_ = slice(j * 512, (j + 1) * 512)
                            nc.tensor.matmul(
                                psA[:, os_],
                                lhsT=kT[0:HD, dot, kt * P : (kt + 1) * P],
                                rhs=qT[0:HD, dot, qs],
                                start=True,
                                stop=True,
                                tile_position=(0, 0),
                            )
                            nc.tensor.matmul(
                                psB[:, os_],
                                lhsT=kT[HD : 2 * HD, dot, kt * P : (kt + 1) * P],
                                rhs=qT[HD : 2 * HD, dot, qs],
                                start=True,
                                stop=True,
                                tile_position=(HD, 0),
                            )
                        for tl, ps, name in ((tA, psA, "A"), (tB, psB, "B")):
                            et = exp_pool.tile(
                                [P, QH], f16, tag="expT", name=f"e{name}_{su}_{kt}"
                            )
                            tl.append(et)
                            nc.scalar.activation(
                                out=et,
                                in_=ps,
                                func=AF.Exp,
                                bias=mask_bias[:, kt : kt + 1],
                                scale=SCALE,
                            )
                    su_state[su] = (tA, tB)

                def emit_tail(su, side):
                    """z + 1/sum + normalize + writeback for one head of su."""
                    dot, qh = su // 2, su % 2
                    h = 2 * dot + side
                    r0 = side * HD
                    q0 = qh * QH
                    tiles = su_state[su][side]

                    zext = work.tile([P, QH], f32, tag="zext")
                    for qc in range(2):
                        ps = psZ.tile([P, 512], f32, tag="z")
                        for kt in range(ST):
                            nc.tensor.matmul(
                                ps[: HD + 1, :],
                                lhsT=vext[:, kt, h, :],
                                rhs=tiles[kt][:, qc * 512 : (qc + 1) * 512],
                                start=(kt == 0),
                                stop=(kt == ST - 1),
                            )
                        nc.vector.tensor_copy(
                            out=zext[: HD + 1, qc * 512 : (qc + 1) * 512],
                            in_=ps[: HD + 1, :],
                        )
                    # 1/sum on DVE in partition layout (no ACT tables):
                    # [1,1024] -> [128,8] -> reciprocal -> DRAM row -> broadcast
                    sum_dram = dram.tile([1, QH], f32, tag="sum_d", bufs=3)
                    nc.sync.dma_start(out=sum_dram, in_=zext[HD : HD + 1, :])
                    rp = work.tile([P, 8], f32, tag="rp")
                    nc.gpsimd.dma_start(
                        out=rp, in_=sum_dram.rearrange("1 (c p) -> p c", p=P)
                    )
                    nc.vector.reciprocal(out=rp, in_=rp)
                    rsum_dram = dram.tile([1, QH], f32, tag="rsum_d", bufs=3)
                    nc.sync.dma_start(
                        out=rsum_dram.rearrange("1 (c p) -> p c", p=P), in_=rp
                    )
                    rbc = work.tile([P, QH], f16, tag="rbc", bufs=3)
                    nc.gpsimd.dma_start(out=rbc, in_=rsum_dram.to_broadcast((P, QH)))

                    for kt in range(ST):
                        nc.vector.tensor_mul(out=tiles[kt], in0=tiles[kt], in1=rbc)
                        nc.sync.dma_start(
                            out=ATTN.ap()[h, kt * P : (kt + 1) * P, q0 : q0 + QH],
                            in_=tiles[kt],
                        )
                    nc.vector.tensor_mul(
                        out=zT[r0 : r0 + HD, dot, q0 : q0 + QH],
                        in0=zext[:HD, :],
                        in1=rbc[:HD, :],
                    )
                    if side == 1:
                        su_state.pop(su)

                def emit_out(dt):
                    """Partial out-projection for head pair dt (zT tile dt)."""
                    for qt in range(ST):
                        ps = psL.tile([P, 1024], f32, tag="l")
                        nc.tensor.matmul(
                            ps[:, :512],
                            lhsT=zT[:, dt, qt * P : (qt + 1) * P],
                            rhs=wo_sb[:, dt, :],
                            start=True,
                            stop=True,
                        )
                        osb = work.tile([P, D], f32, tag="osb")
                        if dt == 0:
                            nc.vector.tensor_add(out=osb, in0=ps[:, :512], in1=wob_bc)
                        else:
                            nc.vector.tensor_copy(out=osb, in_=ps[:, :512])
                        nc.sync.dma_start(
                            out=OUT.ap()[dt, qt * P : (qt + 1) * P, :], in_=osb
                        )

                for su in range(4):
                    emit_logits_exp(su)
                    if su > 0:
                        emit_tail(su - 1, 0)
                        emit_tail(su - 1, 1)
                    if su == 2:
                        emit_out(0)  # heads 0/1 fully done after su=1 tails
                emit_tail(3, 0)
                emit_tail(3, 1)
                emit_out(1)

    nc.finalize()
    return nc


def kernel(Q, K, V, pad_mask, wq_kernel, wq_bias, out_kernel, out_bias, **run_kwargs):
    from concourse.bass_utils import run_bass_kernel_spmd

    if "nc" not in _CACHE:
        _CACHE["nc"] = _build()
    nc = _CACHE["nc"]

    in_maps = []
    for c in range(NCORES):
        b, hg = c // 2, c % 2
        hs = slice(hg * DHG, (hg + 1) * DHG)
        in_maps.append(
            {
                "q_in": np.ascontiguousarray(Q[b], dtype=np.float32),
                "k_in": np.ascontiguousarray(K[b], dtype=np.float32),
                "v_in": np.ascontiguousarray(V[b], dtype=np.float32),
                "mask": np.ascontiguousarray(
                    pad_mask[b, 0, 0, :][None, :], dtype=np.float32
                ),
                "wq": np.ascontiguousarray(wq_kernel[:, hs], dtype=np.float32),
                "wqb": np.ascontiguousarray(wq_bias[hs][None, :], dtype=np.float32),
                "wo": np.ascontiguousarray(out_kernel[hs, :], dtype=np.float32),
                "wob": np.ascontiguousarray(
                    (out_bias if hg == 0 else np.zeros_like(out_bias))[None, :],
                    dtype=np.float32,
                ),
            }
        )

    res = run_bass_kernel_spmd(nc, in_maps, core_ids=list(range(NCORES)), **run_kwargs)
    results = res.results if hasattr(res, "results") else res

    out = np.empty((B, S, D), dtype=np.float32)
    attn = np.empty((B, H, S, S), dtype=np.float32)
    for c in range(NCORES):
        b, hg = c // 2, c % 2
        at = results[c]["attn_t"]  # fp16 [HPC, S(k), S(q)]
        for i in range(HPC):
            attn[b, hg * HPC + i] = at[i].T
    for b in range(B):
        out[b] = (
            results[2 * b]["out_p"].sum(axis=0) + results[2 * b + 1]["out_p"].sum(axis=0)
        )
    if "trace" in run_kwargs:
        _CACHE["last_run"] = res
    return out, attn
